# revision 18
# baseline (speedup 1.0000x reference)
"""GCN (3x GCNConv + mean-pool + linear head) on 8 Trainium2 NeuronCores.

Strategy:
  - dst-nodes partitioned contiguously across 8 cores (6250 each).
  - Node feature table t = D^{-1/2} * h kept in DRAM per layer. t0 is
    input-derived, so every core builds the full t0 table locally (no
    collective). t1/t2 are AllGathered from per-core slices (one collective
    per layer). int16 gather indices are handled by gathering from two base
    offsets of the same table (rows < 32768 / >= 32768).
  - Per layer: dma_gather rows t[src] for this core's edges (grouped by
    dst-window of 128, padded to uniform chunk counts across cores for
    SPMD), scatter-add via {0,1} one-hot (is_equal vs iota) matmuls
    accumulating in PSUM, then per-window epilogue:
      h = relu((dinv * agg) @ W + b),  t_next = dinv * h
  - Mean pool: one-hot over (batch_id - base) + matmul with a ones column
    for counts, dma_scatter_add into a per-core canvas, AllReduce.
  - Head: emb = sums/counts; out = emb @ Wf + bf.
All floating point math runs on device; the host only does integer
index/partitioning prep (bincount degrees, edge bucketing, padding).
"""
import math
import numpy as np

N, E, B, D, H = 50000, 800000, 512, 9, 64
NC = 8
NPC = N // NC            # 6250 nodes per core
NW = 49                  # own windows per core (48 full + 106)
WLAST = NPC - 48 * 128   # 106
GW = (N + 127) // 128    # 391 global windows
NPAD = GW * 128          # 50048
LO = 32768               # rows < LO gathered from base 0; rest from base LO
SEG = 2048               # gather segment (slots per dma_gather call)

_cache = {}


def _wrap16(v, width):
    """Index layout for dma_gather: slot i -> [i%16, i//16], replicated x8."""
    a = np.zeros(((len(v) + 15) // 16) * 16, np.int16)
    a[: len(v)] = v
    a = a.reshape(-1, 16).T.copy()          # [16, n/16]
    if a.shape[1] < width:
        a = np.pad(a, ((0, 0), (0, width - a.shape[1])))
    return np.tile(a, (8, 1))               # [128, n/16]


def _colwrap(v, fill, ncols):
    """[128, ncols] layout: slot i -> [i%128, i//128]."""
    a = np.full(ncols * 128, fill, np.float32)
    a[: len(v)] = v
    return a.reshape(ncols, 128).T.copy()


def _build_host(edge_index, batch):
    src = np.asarray(edge_index[0], np.int64)
    dst = np.asarray(edge_index[1], np.int64)
    sa = np.concatenate([src, np.arange(N, dtype=np.int64)])   # + self loops
    da = np.concatenate([dst, np.arange(N, dtype=np.int64)])
    deg = np.bincount(da, minlength=N).astype(np.float32)      # == ref deg

    owner = da // NPC
    loc = da % NPC
    win = loc // 128
    q = (loc % 128).astype(np.float32)
    stream = (sa >= LO).astype(np.int64)                       # 0 = lo, 1 = hi
    row = np.where(stream == 0, sa, sa - LO).astype(np.int16)

    key = (owner * NW + win) * 2 + stream
    order = np.argsort(key, kind="stable")
    cnt = np.bincount(key, minlength=NC * NW * 2).reshape(NC, NW, 2)
    slots = (np.ceil(cnt.max(axis=0) / 128).astype(np.int64) * 128)  # [NW, 2]
    slots = np.maximum(slots, 128)
    chunks = slots // 128                                       # [NW, 2]
    TA = int(slots[:, 0].sum())
    TB = int(slots[:, 1].sum())
    baseA = np.concatenate([[0], np.cumsum(slots[:, 0])[:-1]])
    baseB = np.concatenate([[0], np.cumsum(slots[:, 1])[:-1]])

    # position of each edge inside its (core, win, stream) bucket
    ks = key[order]
    grp_start = np.concatenate([[0], np.cumsum(np.bincount(ks, minlength=NC * NW * 2))[:-1]])
    rank = np.arange(len(ks)) - grp_start[ks]
    sbase = np.where(stream[order] == 0, baseA[win[order]], baseB[win[order]])
    slot = sbase + rank

    idxA = np.zeros((NC, TA), np.int16)
    qA = np.full((NC, TA), -1.0, np.float32)
    idxB = np.zeros((NC, TB), np.int16)
    qB = np.full((NC, TB), -1.0, np.float32)
    o_, s_, r_, q_, sl_ = owner[order], stream[order], row[order], q[order], slot
    mA = s_ == 0
    idxA[o_[mA], sl_[mA]] = r_[mA]
    qA[o_[mA], sl_[mA]] = q_[mA]
    mB = ~mA
    idxB[o_[mB], sl_[mB]] = r_[mB]
    qB[o_[mB], sl_[mB]] = q_[mB]

    # global degree in window layout [128, GW] (for t0 build)
    degg = np.ones(NPAD, np.float32)
    degg[:N] = deg
    degg = degg.reshape(GW, 128).T.copy()
    # per-core own degree in window layout [128, NW] (for epilogues)
    degw = np.ones((NC, 128 * NW), np.float32)
    degw[:, :NPC] = deg.reshape(NC, NPC)
    degw = degw.reshape(NC, NW, 128).transpose(0, 2, 1).copy()

    # batch metadata
    batch = np.asarray(batch, np.int64)
    bases = batch[np.arange(NC) * NPC]
    bq = np.full((NC, 128 * NW), -1.0, np.float32)
    bq[:, :NPC] = batch.reshape(NC, NPC) - bases[:, None]
    assert bq.max() < 128, "graph span per core exceeds 128"
    bqw = bq.reshape(NC, NW, 128).transpose(0, 2, 1).copy()
    pool_ids = np.empty((NC, 128), np.int64)
    for c in range(NC):
        ids = bases[c] + np.arange(128)
        ids[ids >= 512] = 512 + np.arange(128)[ids >= 512]
        pool_ids[c] = ids

    segsA = [(o, min(SEG, TA - o)) for o in range(0, TA, SEG)]
    segsB = [(o, min(SEG, TB - o)) for o in range(0, TB, SEG)]
    return dict(
        deg=deg, degg=degg, degw=degw, chunks=chunks, TA=TA, TB=TB,
        baseA=baseA, baseB=baseB, idxA=idxA, qA=qA, idxB=idxB, qB=qB,
        bqw=bqw, pool_ids=pool_ids, segsA=segsA, segsB=segsB,
    )


def _build_program(hp):
    import os
    import concourse.bacc as bacc
    import concourse.mybir as mybir
    import concourse.tile as tile

    stage = int(os.environ.get("KSTAGE", "9"))
    ksub = int(os.environ.get("KSUB", "3"))       # 0=gather+AG, 1=+chunks, 3=full
    koh_act = int(os.environ.get("KOH_ACT", "1"))  # 0 = all one-hots on DVE
    krep = int(os.environ.get("KREP", "1"))
    f32, i16 = mybir.dt.float32, mybir.dt.int16
    TA, TB = hp["TA"], hp["TB"]
    chunks = hp["chunks"]
    segsA, segsB = hp["segsA"], hp["segsB"]

    nc = bacc.Bacc("TRN2", target_bir_lowering=False, debug=False, num_devices=NC,
                   num_swdge_queues=4, dynamic_dma_scratch_size=65536)

    # ---- I/O ----
    x64_in = nc.dram_tensor("x64_in", [NPAD, H], f32, kind="ExternalInput").ap()
    degg_in = nc.dram_tensor("degg_in", [128, GW], f32, kind="ExternalInput").ap()
    deg_in = nc.dram_tensor("deg_in", [128, NW], f32, kind="ExternalInput").ap()
    bq_in = nc.dram_tensor("bq_in", [128, NW], f32, kind="ExternalInput").ap()
    iota_in = nc.dram_tensor("iota_in", [128, 128], f32, kind="ExternalInput").ap()
    ident_in = nc.dram_tensor("ident_in", [128, 128], f32, kind="ExternalInput").ap()
    idxA_in = nc.dram_tensor("idxA_in", [128, TA // 16], i16, kind="ExternalInput").ap()
    idxB_in = nc.dram_tensor("idxB_in", [128, TB // 16], i16, kind="ExternalInput").ap()
    qA_in = nc.dram_tensor("qA_in", [128, TA // 128], f32, kind="ExternalInput").ap()
    qB_in = nc.dram_tensor("qB_in", [128, TB // 128], f32, kind="ExternalInput").ap()
    pid_in = nc.dram_tensor("pid_in", [128, 8], i16, kind="ExternalInput").ap()
    W_in = [nc.dram_tensor(f"W{i}_in", [H, H], f32, kind="ExternalInput").ap() for i in range(3)]
    b_in = [nc.dram_tensor(f"b{i}_in", [128, H], f32, kind="ExternalInput").ap() for i in range(3)]
    wf_in = nc.dram_tensor("wf_in", [128, H], f32, kind="ExternalInput").ap()
    bf_in = nc.dram_tensor("bf_in", [128, 1], f32, kind="ExternalInput").ap()

    out_o = nc.dram_tensor("out_o", [B, 1], f32, kind="ExternalOutput").ap()
    emb_o = nc.dram_tensor("emb_o", [B, H], f32, kind="ExternalOutput").ap()
    dbgA_o = nc.dram_tensor("dbgA_o", [128, H], f32, kind="ExternalOutput").ap()
    dbgB_o = nc.dram_tensor("dbgB_o", [128, H], f32, kind="ExternalOutput").ap()

    # internal DRAM tables: t0 fully local; t1/t2 local slice + shared full
    t0_full = nc.dram_tensor("t0_full", [NPAD, H], f32).ap()
    t_loc = [None,
             nc.dram_tensor("t1_loc", [NPC, H], f32).ap(),
             nc.dram_tensor("t2_loc", [NPC, H], f32).ap()]
    t_full = [t0_full,
              nc.dram_tensor("t1_full", [N, H], f32, addr_space="Shared").ap(),
              nc.dram_tensor("t2_full", [N, H], f32, addr_space="Shared").ap()]
    canvas = nc.dram_tensor("canvas", [640, 128], f32).ap()
    canvas_s = nc.dram_tensor("canvas_s", [640, 128], f32, addr_space="Shared").ap()

    RG = [list(range(NC))]

    with tile.TileContext(nc) as tc:
        with tc.tile_pool(name="const", bufs=1) as cpool, \
             tc.tile_pool(name="sbuf", bufs=3) as sbuf, \
             tc.tile_pool(name="stage", bufs=2) as stpool, \
             tc.tile_pool(name="psum", bufs=3, space="PSUM") as pagg, \
             tc.tile_pool(name="psum2", bufs=2, space="PSUM") as paux, \
             tc.tile_pool(name="psum3", bufs=1, space="PSUM") as ppool:

            # ---------- constants ----------
            iota = cpool.tile([128, 128], f32, tag="iota")
            nc.sync.dma_start(out=iota[:], in_=iota_in[:, :])
            ident = cpool.tile([128, 128], f32, tag="ident")
            nc.sync.dma_start(out=ident[:], in_=ident_in[:, :])
            idxA = cpool.tile([128, TA // 16], i16, tag="idxA")
            nc.sync.dma_start(out=idxA[:], in_=idxA_in[:, :])
            idxB = cpool.tile([128, TB // 16], i16, tag="idxB")
            nc.sync.dma_start(out=idxB[:], in_=idxB_in[:, :])
            qA = cpool.tile([128, TA // 128], f32, tag="qA")
            nc.sync.dma_start(out=qA[:], in_=qA_in[:, :])
            qB = cpool.tile([128, TB // 128], f32, tag="qB")
            nc.sync.dma_start(out=qB[:], in_=qB_in[:, :])
            nqA = cpool.tile([128, TA // 128], f32, tag="nqA")
            nc.vector.tensor_scalar_mul(out=nqA[:], in0=qA[:], scalar1=-1.0)
            nqB = cpool.tile([128, TB // 128], f32, tag="nqB")
            nc.vector.tensor_scalar_mul(out=nqB[:], in0=qB[:], scalar1=-1.0)
            bq = cpool.tile([128, NW], f32, tag="bq")
            nc.sync.dma_start(out=bq[:], in_=bq_in[:, :])
            pid = cpool.tile([128, 8], i16, tag="pid")
            nc.sync.dma_start(out=pid[:], in_=pid_in[:, :])
            Wt = []
            for i in range(3):
                w = cpool.tile([H, H], f32, tag=f"W{i}")
                nc.sync.dma_start(out=w[:], in_=W_in[i][:, :])
                Wt.append(w)
            bt = []
            for i in range(3):
                b_ = cpool.tile([128, H], f32, tag=f"b{i}")
                nc.sync.dma_start(out=b_[:], in_=b_in[i][:, :])
                bt.append(b_)
            wf = cpool.tile([128, H], f32, tag="wf")
            nc.sync.dma_start(out=wf[:], in_=wf_in[:, :])
            bf = cpool.tile([128, 1], f32, tag="bf")
            nc.sync.dma_start(out=bf[:], in_=bf_in[:, :])

            # global dinv [128, GW] (t0 build) and own dinv [128, NW] (epilogues)
            degg_t = cpool.tile([128, GW], f32, tag="degg")
            nc.sync.dma_start(out=degg_t[:], in_=degg_in[:, :])
            dsqg = cpool.tile([128, GW], f32, tag="dsqg")
            nc.scalar.activation(out=dsqg[:], in_=degg_t[:], func=mybir.ActivationFunctionType.Sqrt)
            dinvg = cpool.tile([128, GW], f32, tag="dinvg")
            nc.vector.reciprocal(out=dinvg[:], in_=dsqg[:])

            deg_t = cpool.tile([128, NW], f32, tag="deg")
            nc.sync.dma_start(out=deg_t[:], in_=deg_in[:, :])
            dsq = cpool.tile([128, NW], f32, tag="dsq")
            nc.scalar.activation(out=dsq[:], in_=deg_t[:], func=mybir.ActivationFunctionType.Sqrt)
            dinv = cpool.tile([128, NW], f32, tag="dinv")
            nc.vector.reciprocal(out=dinv[:], in_=dsq[:])

            for rep_i in range(krep):
                # ---------- t0 = dinv * x, built fully local ----------
                TSEG = 17                      # windows per t0 segment
                for s0 in range(0, GW, TSEG):
                    s1 = min(s0 + TSEG, GW)
                    nwin = s1 - s0
                    xw = sbuf.tile([128, TSEG, H], f32, tag="xw")
                    nc.sync.dma_start(
                        out=xw[:, :nwin, :],
                        in_=x64_in[s0 * 128 : s1 * 128, :].rearrange("(w p) f -> p w f", p=128),
                    )
                    t0t = sbuf.tile([128, TSEG, H], f32, tag="t0t")
                    for w in range(nwin):
                        nc.vector.tensor_scalar_mul(
                            out=t0t[:, w, :], in0=xw[:, w, :],
                            scalar1=dinvg[:, s0 + w : s0 + w + 1],
                        )
                    nc.sync.dma_start(
                        out=t0_full[s0 * 128 : s1 * 128, :].rearrange("(w p) f -> p w f", p=128),
                        in_=t0t[:, :nwin, :],
                    )

                def store_and_allgather(stt, l):
                    nc.sync.dma_start(
                        out=t_loc[l][0 : 48 * 128, :].rearrange("(w p) f -> p w f", p=128),
                        in_=stt[:, 0:48, :],
                    )
                    nc.sync.dma_start(
                        out=t_loc[l][48 * 128 : NPC, :],
                        in_=stt[:WLAST, NW - 1, :],
                    )
                    nc.gpsimd.collective_compute(
                        "AllGather", mybir.AluOpType.bypass, replica_groups=RG,
                        ins=[t_loc[l][:, :]], outs=[t_full[l][:, :]],
                    )

                h3st = cpool.tile([128, NW, H + 1], f32, tag="h3st")
                nc.vector.memset(h3st[:, :, :], 1.0)

                # ---------- layers ----------
                def onehot_build(k, qt, nqt, col):
                    if koh_act == 0 or k % 3 < 2:
                        oh = sbuf.tile([128, 128], f32, tag="oh")
                        nc.vector.tensor_scalar(
                            out=oh[:], in0=iota[:], scalar1=qt[:, col : col + 1],
                            scalar2=None, op0=mybir.AluOpType.is_equal,
                        )
                    else:
                        t1 = sbuf.tile([128, 128], f32, tag="oht")
                        nc.scalar.activation(
                            out=t1[:], in_=iota[:], func=mybir.ActivationFunctionType.Square,
                            bias=nqt[:, col : col + 1],
                        )
                        oh = sbuf.tile([128, 128], f32, tag="oh")
                        nc.scalar.activation(
                            out=oh[:], in_=t1[:], func=mybir.ActivationFunctionType.Relu,
                            bias=1.0, scale=-1.0,
                        )
                    return oh

                for l in range(min(3, stage)):
                    tin = t_full[l]
                    n_hi = int(tin.shape[0]) - LO
                    gq = 0
                    stA = []
                    for (off, sz) in segsA:
                        st = stpool.tile([128, SEG // 128, H], f32, tag="stA")
                        nc.gpsimd.dma_gather(
                            out_ap=st[:, : sz // 128, :], in_ap=tin[0:LO, :],
                            idxs_ap=idxA[:, off // 16 : (off + sz) // 16],
                            num_idxs=sz, num_idxs_reg=sz, elem_size=H,
                            single_packet=False, queue_num=gq % 4,
                        )
                        gq += 1
                        stA.append(st)
                    stB = []
                    for (off, sz) in segsB:
                        st = stpool.tile([128, SEG // 128, H], f32, tag="stB")
                        nc.gpsimd.dma_gather(
                            out_ap=st[:, : sz // 128, :], in_ap=tin[LO : LO + n_hi, :],
                            idxs_ap=idxB[:, off // 16 : (off + sz) // 16],
                            num_idxs=sz, num_idxs_reg=sz, elem_size=H,
                            single_packet=False, queue_num=gq % 4,
                        )
                        gq += 1
                        stB.append(st)

                    k = 0
                    tst = None
                    if l < 2:
                        tst = cpool.tile([128, NW, H], f32, tag="tst")
                    for w in range(NW):
                        if ksub == 0:
                            if tst is not None:
                                nc.vector.memset(tst[:, w, :], 0.0)
                            continue
                        nchunks = int(chunks[w, 0] + chunks[w, 1])
                        agg = pagg.tile([128, H], f32, tag="agg")
                        ci = 0
                        for s in range(2):
                            base = (hp["baseA"] if s == 0 else hp["baseB"])[w]
                            qt = qA if s == 0 else qB
                            nqt = nqA if s == 0 else nqB
                            stages = stA if s == 0 else stB
                            for j in range(int(chunks[w, s])):
                                slot0 = int(base) + j * 128
                                oh = onehot_build(k, qt, nqt, slot0 // 128)
                                k += 1
                                seg_i, seg_c = slot0 // SEG, (slot0 % SEG) // 128
                                nc.tensor.matmul(
                                    out=agg[:], lhsT=oh[:],
                                    rhs=stages[seg_i][:, seg_c, :],
                                    start=(ci == 0), stop=(ci == nchunks - 1),
                                )
                                ci += 1
                        if ksub == 1:
                            if tst is not None:
                                nc.vector.tensor_copy(out=tst[:, w, :], in_=agg[:])
                            else:
                                dmp = sbuf.tile([128, H], f32, tag="u")
                                nc.vector.tensor_copy(out=dmp[:], in_=agg[:])
                                nc.scalar.activation(
                                    out=h3st[:, w, 0:H], in_=dmp[:],
                                    func=mybir.ActivationFunctionType.Relu,
                                )
                            continue
                        # epilogue: h = relu(dinv*agg @ W + b); t_next = dinv*h
                        u = sbuf.tile([128, H], f32, tag="u")
                        nc.vector.tensor_scalar_mul(
                            out=u[:], in0=agg[:], scalar1=dinv[:, w : w + 1]
                        )
                        tp = paux.tile([H, 128], f32, tag="tp")
                        nc.tensor.transpose(out=tp[:], in_=u[:], identity=ident[:])
                        s2 = sbuf.tile([H, 128], f32, tag="s2")
                        nc.vector.tensor_copy(out=s2[:], in_=tp[:])
                        hps = paux.tile([128, H], f32, tag="hp")
                        nc.tensor.matmul(
                            out=hps[:], lhsT=s2[:], rhs=Wt[l][:], start=True, stop=True
                        )
                        hb = sbuf.tile([128, H], f32, tag="hb")
                        nc.vector.tensor_tensor(
                            out=hb[:], in0=hps[:], in1=bt[l][:], op=mybir.AluOpType.add
                        )
                        if l < 2:
                            hr = sbuf.tile([128, H], f32, tag="hr")
                            nc.scalar.activation(
                                out=hr[:], in_=hb[:], func=mybir.ActivationFunctionType.Relu
                            )
                            nc.vector.tensor_scalar_mul(
                                out=tst[:, w, :], in0=hr[:], scalar1=dinv[:, w : w + 1]
                            )
                        else:
                            nc.scalar.activation(
                                out=h3st[:, w, 0:H], in_=hb[:],
                                func=mybir.ActivationFunctionType.Relu,
                            )
                    if l < 2:
                        store_and_allgather(tst, l + 1)

                # ---------- debug taps ----------
                Lc = min(stage, 2)
                dbga = sbuf.tile([128, H], f32, tag="dbga")
                nc.sync.dma_start(out=dbga[:], in_=t_full[Lc][0:128, :])
                nc.sync.dma_start(out=dbgA_o[:, :], in_=dbga[:])
                dbgb = sbuf.tile([128, H], f32, tag="dbgb")
                nc.sync.dma_start(out=dbgb[:], in_=t_full[Lc][LO : LO + 128, :])
                nc.sync.dma_start(out=dbgB_o[:, :], in_=dbgb[:])
                if stage < 4:
                    zz = sbuf.tile([128, H], f32, tag="zz")
                    nc.vector.memset(zz[:], 0.0)
                    for g in range(4):
                        nc.sync.dma_start(out=emb_o[g * 128 : (g + 1) * 128, :], in_=zz[:])
                        nc.sync.dma_start(out=out_o[g * 128 : (g + 1) * 128, :], in_=zz[:, 0:1])

                if stage >= 4:
                    # ---------- mean pool ----------
                    pps = ppool.tile([128, 128], f32, tag="pool")
                    for w in range(NW):
                        ohb = sbuf.tile([128, 128], f32, tag="oh")
                        nc.vector.tensor_scalar(
                            out=ohb[:], in0=iota[:], scalar1=bq[:, w : w + 1],
                            scalar2=None, op0=mybir.AluOpType.is_equal,
                        )
                        nc.tensor.matmul(
                            out=pps[:, 0 : H + 1], lhsT=ohb[:], rhs=h3st[:, w, :],
                            start=(w == 0), stop=(w == NW - 1),
                        )
                    val = sbuf.tile([128, 128], f32, tag="val")
                    nc.vector.memset(val[:], 0.0)
                    nc.vector.tensor_copy(out=val[:, 0 : H + 1], in_=pps[:, 0 : H + 1])
                    zt = sbuf.tile([128, 128], f32, tag="zt")
                    nc.vector.memset(zt[:], 0.0)
                    for bb in range(5):
                        nc.sync.dma_start(out=canvas[bb * 128 : (bb + 1) * 128, :], in_=zt[:])
                    nc.gpsimd.dma_scatter_add(
                        out_ap=canvas[:, :],
                        in_ap=val[:, :].rearrange("p (a f) -> p a f", a=1),
                        idxs_ap=pid[:, :], num_idxs=128, num_idxs_reg=128, elem_size=128,
                    )
                    nc.gpsimd.collective_compute(
                        "AllReduce", mybir.AluOpType.add, replica_groups=RG,
                        ins=[canvas[:, :]], outs=[canvas_s[:, :]],
                    )

                    # ---------- head ----------
                    for g in range(4):
                        cw = sbuf.tile([128, 128], f32, tag="cw")
                        nc.sync.dma_start(out=cw[:], in_=canvas_s[g * 128 : (g + 1) * 128, :])
                        c1 = sbuf.tile([128, 1], f32, tag="c1")
                        nc.vector.tensor_scalar_max(out=c1[:], in0=cw[:, H : H + 1], scalar1=1.0)
                        rc = sbuf.tile([128, 1], f32, tag="rc")
                        nc.vector.reciprocal(out=rc[:], in_=c1[:])
                        ew = sbuf.tile([128, H], f32, tag="ew")
                        nc.vector.tensor_scalar_mul(out=ew[:], in0=cw[:, 0:H], scalar1=rc[:, 0:1])
                        nc.sync.dma_start(out=emb_o[g * 128 : (g + 1) * 128, :], in_=ew[:])
                        pr = sbuf.tile([128, H], f32, tag="pr")
                        nc.vector.tensor_tensor(out=pr[:], in0=ew[:], in1=wf[:], op=mybir.AluOpType.mult)
                        oc = sbuf.tile([128, 1], f32, tag="oc")
                        nc.vector.tensor_reduce(
                            out=oc[:], in_=pr[:], axis=mybir.AxisListType.X, op=mybir.AluOpType.add
                        )
                        oc2 = sbuf.tile([128, 1], f32, tag="oc2")
                        nc.vector.tensor_tensor(out=oc2[:], in0=oc[:], in1=bf[:], op=mybir.AluOpType.add)
                        nc.sync.dma_start(out=out_o[g * 128 : (g + 1) * 128, :], in_=oc2[:])

    nc.finalize()
    return nc


def _build_in_maps(hp, x, W1, b1, W2, b2, W3, b3, Wf, bf):
    x = np.asarray(x, np.float32)
    x64 = np.zeros((NPAD, H), np.float32)
    x64[:N, :D] = x
    iota = np.tile(np.arange(128, dtype=np.float32), (128, 1))
    ident = np.eye(128, dtype=np.float32)
    W1p = np.zeros((H, H), np.float32)
    W1p[:D] = np.asarray(W1, np.float32)
    Ws = [W1p, np.asarray(W2, np.float32), np.asarray(W3, np.float32)]
    bs = [np.tile(np.asarray(v, np.float32)[None, :], (128, 1)) for v in (b1, b2, b3)]
    wfb = np.tile(np.asarray(Wf, np.float32)[:, 0][None, :], (128, 1))
    bfb = np.full((128, 1), np.float32(np.asarray(bf, np.float32)[0]), np.float32)

    in_maps = []
    for c in range(NC):
        m = {
            "x64_in": x64,
            "degg_in": hp["degg"],
            "deg_in": hp["degw"][c],
            "bq_in": hp["bqw"][c],
            "iota_in": iota,
            "ident_in": ident,
            "idxA_in": _wrap16(hp["idxA"][c], hp["TA"] // 16),
            "idxB_in": _wrap16(hp["idxB"][c], hp["TB"] // 16),
            "qA_in": _colwrap(hp["qA"][c], -1.0, hp["TA"] // 128),
            "qB_in": _colwrap(hp["qB"][c], -1.0, hp["TB"] // 128),
            "pid_in": _wrap16(hp["pool_ids"][c].astype(np.int16), 8),
        }
        for i in range(3):
            m[f"W{i}_in"] = Ws[i]
            m[f"b{i}_in"] = bs[i]
        m["wf_in"] = wfb
        m["bf_in"] = bfb
        in_maps.append(m)
    return in_maps


def bench(x, edge_index, batch, W1, b1, W2, b2, W3, b3, Wf, bf, iters=20, prog=None):
    """Jit once, device-resident inputs, wall times of repeated executes."""
    import time
    import jax
    import numpy as np
    from jax.sharding import Mesh, PartitionSpec, NamedSharding
    from jax.experimental.shard_map import shard_map
    import concourse.mybir as mb
    from concourse import bass2jax

    if prog is not None:
        hp, nc = prog
    else:
        ekey = (hash(np.asarray(edge_index).tobytes()), hash(np.asarray(batch).tobytes()))
        if ekey not in _cache:
            hp = _build_host(edge_index, batch)
            nc = _build_program(hp)
            _cache.clear()
            _cache[ekey] = (hp, nc)
        hp, nc = _cache[ekey]
    in_maps = _build_in_maps(hp, x, W1, b1, W2, b2, W3, b3, Wf, bf)

    bass2jax.install_neuronx_cc_hook()
    partition_name = nc.partition_id_tensor.name if nc.partition_id_tensor else None
    in_names, out_names, out_avals, zero_outs = [], [], [], []
    for alloc in nc.m.functions[0].allocations:
        if not isinstance(alloc, mb.MemoryLocationSet):
            continue
        name = alloc.memorylocations[0].name
        if alloc.kind == "ExternalInput":
            if name != partition_name:
                in_names.append(name)
        elif alloc.kind == "ExternalOutput":
            out_names.append(name)
            shape = tuple(alloc.tensor_shape)
            dtype = mb.dt.np(alloc.dtype)
            out_avals.append(jax.core.ShapedArray(shape, dtype))
            zero_outs.append(np.zeros(shape, dtype))
    n_params = len(in_names)
    in_names_all = in_names + out_names
    if partition_name is not None:
        in_names_all.append(partition_name)

    def _body(*args):
        operands = list(args)
        if partition_name is not None:
            operands.append(bass2jax.partition_id_tensor())
        outs = bass2jax._bass_exec_p.bind(
            *operands,
            out_avals=tuple(out_avals),
            in_names=tuple(in_names_all),
            out_names=tuple(out_names),
            lowering_input_output_aliases=(),
            sim_require_finite=True,
            sim_require_nnan=True,
            nc=nc,
        )
        return tuple(outs)

    devices = jax.devices()[:NC]
    mesh = Mesh(np.asarray(devices), ("core",))
    nouts = len(out_names)
    in_specs = (PartitionSpec("core"),) * (n_params + nouts)
    out_specs = (PartitionSpec("core"),) * nouts
    fn = jax.jit(shard_map(_body, mesh=mesh, in_specs=in_specs, out_specs=out_specs, check_rep=False))

    sh = NamedSharding(mesh, PartitionSpec("core"))
    dev_in = [
        jax.device_put(np.concatenate([np.asarray(in_maps[c][n]) for c in range(NC)], axis=0), sh)
        for n in in_names
    ]
    dev_zero = [
        jax.device_put(np.concatenate([z] * NC, axis=0), sh) for z in zero_outs
    ]
    r = fn(*dev_in, *dev_zero)
    jax.block_until_ready(r)
    times = []
    for _ in range(iters):
        t0 = time.perf_counter()
        r = fn(*dev_in, *dev_zero)
        jax.block_until_ready(r)
        times.append(time.perf_counter() - t0)
    return times


class _SimResults:
    def __init__(self, results):
        self.results = results
        self.exec_time_ns = None
        self.instructions_and_trace = None


last_result = None


def kernel(x, edge_index, batch, W1, b1, W2, b2, W3, b3, Wf, bf, trace=False, sim=False):
    global last_result
    from concourse.bass_utils import run_bass_kernel_spmd

    ekey = (hash(np.asarray(edge_index).tobytes()), hash(np.asarray(batch).tobytes()))
    if ekey not in _cache:
        hp = _build_host(edge_index, batch)
        nc = _build_program(hp)
        _cache.clear()
        _cache[ekey] = (hp, nc)
    hp, nc = _cache[ekey]

    in_maps = _build_in_maps(hp, x, W1, b1, W2, b2, W3, b3, Wf, bf)
    if sim:
        from concourse.bass_interp import MultiCoreSim

        msim = MultiCoreSim(nc, num_cores=NC, require_finite=False, require_nnan=False)
        cores = list(msim.cores.values())
        for c, core in enumerate(cores):
            for k, v in in_maps[c].items():
                core.tensor(k)[:] = v
        msim.simulate(check_with_hw=False)
        outs = ["out_o", "emb_o", "dbgA_o", "dbgB_o"]
        res = _SimResults([{k: np.array(core.tensor(k)) for k in outs} for core in cores])
    else:
        res = run_bass_kernel_spmd(nc, in_maps, core_ids=list(range(NC)), trace=trace)
    last_result = res
    r0 = res.results[0]
    return np.asarray(r0["out_o"]), np.asarray(r0["emb_o"])
